# revision 1
# baseline (speedup 1.0000x reference)
"""Trainium2 Bass kernel for CondLaneRNNHead-style dynamic mask head.

Computation (see reference): per-instance 3-layer 1x1-conv MLP over
per-image feature maps augmented with 2 coordinate channels.

  out[m] = w2[m] @ relu(w1[m] @ relu(w0[m] @ [coords; x[img(m)]] + b0[m]) + b1[m]) + b2[m]

Shapes: x [4, 64, 80, 200] f32, mask_head_params [32, 8513] f32, num_ins=8.
Output [1, 32, 80, 200] f32.

Sharding: spatial, along H. Core k processes rows [10k, 10k+10) of all 4
images for all 32 instances. This replicates only the (small) per-instance
params across cores; the big x tensor is read exactly once in aggregate.

Device kernel structure (per core):
  - feats[img] SBUF tile [66, 2000]: partitions 0-63 = x channels,
    partitions 64-65 = (xx/W, yy/W) coordinate rows.
  - Instances are packed in PAIRS (2 instances of the same image):
      layer0: lhsT [66, 128]  (cols 0-63 inst a, 64-127 inst b), one matmul
              computes both instances' 64 hidden channels.
      layer1: lhsT [128, 128] block-diagonal (w1a.T | w1b.T).
      layer2: lhsT [128, 2]   ([w2a;0] | [0;w2b]).
    Matmuls run in bf16 (fp32 PSUM accumulate). fp32r was measured 5x
    slower: fp32 weights get no fast-weight-load and no background weight
    buffer, so every matmul serialized a ~300ns LDWEIGHTS and paid isolated
    fill+drain latency (~880ns/matmul vs ~220ns bf16 warm).
  - ReLU+bias layer0 on ScalarE (activation, PSUM->SBUF); layer1 on VectorE
    (tensor_scalar add+max, PSUM->SBUF) to balance engines.
  - layer2 outputs [2, free] of 4 pairs (a "quad" = 8 instances, one image)
    are packed into ONE PSUM tile at partition offsets {0,32,64,96} (matmul
    col tile positions), so the PSUM->SBUF move + b2 bias is a single
    [128, 2000] op per quad instead of 16 thin [2, .] ops. DMA cannot read
    PSUM on trn2, and ACT/DVE op cost is free-dim-driven (partition count
    free), so dense partition packing is what makes the move cheap.
"""

import numpy as np
from contextlib import ExitStack

N_IMG, C, H, W = 4, 64, 80, 200
NUM_INS = 8
M = N_IMG * NUM_INS          # 32 instances
N_CORES = 8
HPC = H // N_CORES           # 10 rows of H per core
SPI = HPC * W                # 2000 spatial positions per image slice
PAIRS = M // 2               # 16
CH = C + 2                   # 66 input channels incl. coords
FD = 1000                    # activation chunk
# matmul free-dim splits inside each 1000 chunk: PSUM banks hold 512 f32, and
# a matmul output must not cross a bank boundary -> split 512 + 488.
SPLITS = ((0, 512), (512, 488))
FDP = 1024                   # padded per-half stride in the quad PSUM tile

_W0N, _W1N, _W2N = CH * C, C * C, C
_B2_SHIFT = -2.19

_COMPILED = {}


def _build_program():
    import concourse.bacc as bacc
    import concourse.tile as tile
    from concourse import mybir

    dt = mybir.dt
    AF = mybir.ActivationFunctionType
    OP = mybir.AluOpType

    nc = bacc.Bacc("TRN2", target_bir_lowering=False, debug=False)

    # xs packs the 2 coordinate rows below the 64 x-channels so each image's
    # feats tile is filled by a single DMA (matmuls tolerate few sync waits).
    xs_d = nc.dram_tensor("xs", [N_IMG, CH, SPI], dt.bfloat16, kind="ExternalInput").ap()
    # layer0 lhsT zero-padded to K=128: K=66 matmuls light up only half the
    # PE rows, which keeps the HAM activity monitor below its un-throttle
    # threshold (PE then runs at 1.2 instead of 2.4 GHz).
    l0_d = nc.dram_tensor("l0t", [128, PAIRS * 128], dt.bfloat16, kind="ExternalInput").ap()
    l1_d = nc.dram_tensor("l1t", [128, PAIRS * 128], dt.bfloat16, kind="ExternalInput").ap()
    # layer2 runs in bf16: fp32r matmuls require dst start_partition == 0,
    # which the quad partition-packing (offsets 32/64/96) violates.
    # lhsT cols 2-31 are zeros: each mm2 then writes a full 32-partition
    # group, keeping PSUM fully initialized at no PE cost (time ~ free size).
    l2_d = nc.dram_tensor("l2t", [128, PAIRS * 32], dt.bfloat16, kind="ExternalInput").ap()
    b0_d = nc.dram_tensor("b0t", [128, PAIRS], dt.float32, kind="ExternalInput").ap()
    b1_d = nc.dram_tensor("b1t", [128, PAIRS], dt.float32, kind="ExternalInput").ap()
    b2_d = nc.dram_tensor("b2q", [128, 4], dt.float32, kind="ExternalInput").ap()
    # out[q, j, r, :] = instance 8q + 2j + r, i.e. plain instance-major order
    out_d = nc.dram_tensor("out", [4, 4, 2, SPI], dt.float32, kind="ExternalOutput").ap()

    f32 = dt.float32
    bf16 = dt.bfloat16

    with tile.TileContext(nc) as tc, ExitStack() as ctx:
        cpool = ctx.enter_context(tc.tile_pool(name="const", bufs=1))
        hpool = ctx.enter_context(tc.tile_pool(name="work", bufs=4))
        pspool = ctx.enter_context(tc.tile_pool(name="ps", bufs=3, space="PSUM"))
        psqpool = ctx.enter_context(tc.tile_pool(name="psq", bufs=1, space="PSUM"))

        # ---- resident tiles + loads ----
        # Small bias/weight tensors first: the sync sequencer issues DMAs in
        # order (~0.6us each), and the first ReLU needs b0s — emitting biases
        # last cost a ~20us pipeline stall at the head.
        b0s = cpool.tile([128, PAIRS], f32, tag="b0s", name="b0s")
        nc.sync.dma_start(b0s[:], b0_d[:])
        b1s = cpool.tile([128, PAIRS], f32, tag="b1s", name="b1s")
        nc.sync.dma_start(b1s[:], b1_d[:])
        b2s = cpool.tile([128, 4], f32, tag="b2s", name="b2s")
        nc.sync.dma_start(b2s[:], b2_d[:])
        l2s = cpool.tile([128, PAIRS * 32], bf16, tag="l2s", name="l2s")
        nc.sync.dma_start(l2s[:], l2_d[:])
        fe = []
        l0s = []
        l1s = []
        for n in range(N_IMG):
            t = cpool.tile([128, SPI], bf16, tag=f"fe{n}", name=f"fe{n}")
            # memset first (engines need 32-aligned partition starts); the
            # coord-row DMA below then overwrites rows 64-65
            nc.gpsimd.memset(t[64:128, :], 0.0)
            # split across partition chunks so the transfer spreads over
            # multiple DMA engines (~23 GB/s each)
            for a, b in ((0, 17), (17, 34), (34, 50), (50, CH)):
                nc.sync.dma_start(t[a:b, :], xs_d[n, a:b, :])
            fe.append(t)
            g0 = cpool.tile([128, 4 * 128], bf16, tag=f"l0g{n}", name=f"l0g{n}")
            nc.sync.dma_start(g0[:], l0_d[:, n * 512 : (n + 1) * 512])
            l0s.append(g0)
            g1 = cpool.tile([128, 4 * 128], bf16, tag=f"l1g{n}", name=f"l1g{n}")
            nc.sync.dma_start(g1[:], l1_d[:, n * 512 : (n + 1) * 512])
            l1s.append(g1)

        # ---- PE warmup on a zeroed dummy tile ----
        # Runs during the input-DMA head (no data deps), attempting to lift
        # the HAM clock gate (1.2 -> 2.4 GHz) before the real stream; costs
        # nothing even if the gate stays cold since it overlaps the DMAs.
        wsrc = cpool.tile([128, 640], bf16, tag="wsrc", name="wsrc")
        nc.gpsimd.memset(wsrc[:], 0.0)
        # dummy Relu so the ACT table-set DMA issues at t~0 instead of
        # queueing behind the input DMAs (measured 22us first-ReLU stall)
        wact = cpool.tile([128, 8], f32, tag="wact", name="wact")
        nc.scalar.activation(wact[:], wsrc[:, 0:8], AF.Relu, bias=0.0)
        wps = pspool.tile([128, FD], f32, tag="ps", name="wps")
        for _ in range(44):
            nc.tensor.matmul(
                wps[:, 0:512], wsrc[:, 0:128], wsrc[:, 128:640],
                start=True, stop=True,
            )

        # ---- main loop: quads of pairs (8 instances of one image) ----
        for q in range(4):
            img = q
            for hh in range(SPI // FD):
                base = hh * FD
                psq = psqpool.tile([128, FDP], f32, tag="psq", name="psq")
                # during the pipeline ramp (first chains in flight) the PE
                # sits idle between a pair's layers; in-order execution means
                # only instructions placed THERE can fill the gap. These
                # zero-matmuls stomp a freshly allocated PSUM tile BEFORE its
                # real start=True matmuls overwrite it, keeping HAM activity
                # up so the 2.4 GHz clock survives the ramp.
                def _fill(dst, n_mm):
                    for _ in range(n_mm):
                        nc.tensor.matmul(
                            dst[:, 0:512], wsrc[:, 0:128], wsrc[:, 128:640],
                            start=True, stop=True,
                        )

                ramp = q == 0 and hh == 0
                for j in range(4):
                    p = 4 * q + j
                    w0 = l0s[img][:, j * 128 : (j + 1) * 128]
                    w1 = l1s[img][:, j * 128 : (j + 1) * 128]
                    w2 = l2s[:, 32 * p : 32 * p + 32]
                    ps0 = pspool.tile([128, FD], f32, tag="ps", name="ps0")
                    if ramp and j < 3:
                        _fill(ps0, 4 - j)
                    for off, sz in SPLITS:
                        nc.tensor.matmul(
                            ps0[:, off : off + sz],
                            w0,
                            fe[img][:, base + off : base + off + sz],
                            start=True,
                            stop=True,
                        )
                    h1 = hpool.tile([128, FD], bf16, tag="h1", name="h1")
                    if p % 2 == 0:
                        nc.scalar.activation(
                            h1[:], ps0[:], AF.Relu, bias=b0s[:, p : p + 1]
                        )
                    else:
                        nc.vector.tensor_scalar(
                            h1[:], ps0[:], b0s[:, p : p + 1], 0.0, OP.add, OP.max
                        )
                    ps1 = pspool.tile([128, FD], f32, tag="ps", name="ps1")
                    if ramp and j < 3:
                        _fill(ps1, 4 - j)
                    for off, sz in SPLITS:
                        nc.tensor.matmul(
                            ps1[:, off : off + sz],
                            w1,
                            h1[:, off : off + sz],
                            start=True,
                            stop=True,
                        )
                    h2 = hpool.tile([128, FD], bf16, tag="h2", name="h2")
                    if p % 2 == 0:
                        nc.vector.tensor_scalar(
                            h2[:], ps1[:], b1s[:, p : p + 1], 0.0, OP.add, OP.max
                        )
                    else:
                        nc.scalar.activation(
                            h2[:], ps1[:], AF.Relu, bias=b1s[:, p : p + 1]
                        )
                    for off, sz in SPLITS:
                        nc.tensor.matmul(
                            psq[32 * j : 32 * j + 32, off : off + sz],
                            w2,
                            h2[:, off : off + sz],
                            start=True,
                            stop=True,
                            tile_position=(0, 32 * j),
                        )
                # fused bias+move for the (quad, half); alternate engines
                oq = hpool.tile([128, FD], f32, tag="oq", name="oq", bufs=6)
                if (2 * q + hh) % 2 == 0:
                    nc.scalar.activation(
                        oq[:], psq[:, 0:FD], AF.Identity, bias=b2s[:, q : q + 1]
                    )
                else:
                    nc.vector.tensor_scalar(
                        oq[:], psq[:, 0:FD], b2s[:, q : q + 1], None, OP.add
                    )
                # contiguous 2-row reads (strided-partition reads miss deps
                # in the tile tracker and raced the move op)
                # issue output DMAs from the otherwise-idle gpsimd queue so
                # they don't serialize behind input DMAs on the sync engine
                for j in range(4):
                    nc.gpsimd.dma_start(
                        out_d[q, j, :, base : base + FD], oq[32 * j : 32 * j + 2, :]
                    )

    nc.compile()
    _dedupe_ldweights(nc, mybir)
    return nc


def _dedupe_ldweights(nc, mybir):
    """Drop redundant PE LDWEIGHTS after compile.

    Tile emits one LDWEIGHTS per matmul; consecutive matmuls here often share
    one stationary operand (split-column pairs, the warmup burst), so the
    repeat loads only serialize the PE (~120ns each, and they block drain/fill
    overlap between back-to-back matmuls). Safe removal criteria: identical
    weights AP + tile_position as the last retained LDWEIGHTS (nothing between
    two LDWEIGHTS changes the loaded weights), and no semaphore waits/updates
    on the dropped instruction, so synchronization is untouched.
    """
    dropped = 0
    for fn in nc.m.functions:
        for blk in fn.blocks:
            new = []
            last_sig = None
            for i in blk.instructions:
                if (
                    isinstance(i, mybir.InstLdweights)
                    and i.engine == mybir.EngineType.PE
                ):
                    sig = (
                        str(i.ins[0]),
                        tuple(i.tile_position or ()),
                        i.perf_mode,
                        i.is_transpose,
                    )
                    si = i.sync_info
                    clean = si is None or (not si.on_wait and not si.on_update)
                    if clean and sig == last_sig:
                        dropped += 1
                        continue
                    last_sig = sig
                new.append(i)
            if dropped:
                blk.instructions.clear()
                blk.instructions.extend(new)
    return dropped


def _pack_params(mask_head_params):
    """Split generated params and build the pair-packed device layouts."""
    p = np.ascontiguousarray(mask_head_params, dtype=np.float32)
    o0, o1, o2 = _W0N, _W0N + _W1N, _W0N + _W1N + _W2N
    w0 = p[:, :o0].reshape(M, C, CH)
    w1 = p[:, o0:o1].reshape(M, C, C)
    w2 = p[:, o1:o2].reshape(M, C)
    b0 = p[:, o2 : o2 + C]
    b1 = p[:, o2 + C : o2 + 2 * C]
    b2 = p[:, o2 + 2 * C :] + np.float32(_B2_SHIFT)

    import ml_dtypes as _mld

    # layer0 lhsT [128, 16*128]: rows 0-63 = x-channel weights, 64-65 = coord
    # weights, 66-127 zero K-pad; cols pair-major then (inst a | inst b).
    w0T = np.transpose(w0, (2, 0, 1))              # [66(cin), 32, 64]
    w0T = np.concatenate([w0T[2:], w0T[:2]], 0)    # x channels first, coords last
    l0t = np.zeros((128, M * C), dtype=np.float32)
    l0t[:CH] = w0T.reshape(CH, M * C)
    l0t = np.ascontiguousarray(l0t.astype(_mld.bfloat16))

    l1 = np.zeros((PAIRS, 128, 128), dtype=np.float32)
    l1[:, :C, :C] = np.transpose(w1[0::2], (0, 2, 1))
    l1[:, C:, C:] = np.transpose(w1[1::2], (0, 2, 1))
    l1t = np.ascontiguousarray(
        np.transpose(l1, (1, 0, 2)).reshape(128, PAIRS * 128).astype(_mld.bfloat16)
    )

    import ml_dtypes
    l2 = np.zeros((PAIRS, 128, 32), dtype=np.float32)
    l2[:, :C, 0] = w2[0::2]
    l2[:, C:, 1] = w2[1::2]
    l2t = np.ascontiguousarray(
        np.transpose(l2, (1, 0, 2)).reshape(128, PAIRS * 32).astype(ml_dtypes.bfloat16)
    )

    b0t = np.ascontiguousarray(np.concatenate([b0[0::2], b0[1::2]], 1).T)  # [128,16]
    b1t = np.ascontiguousarray(np.concatenate([b1[0::2], b1[1::2]], 1).T)
    # b2 packed to match the quad PSUM layout: rows 32j+r of col q hold
    # instance 8q + 2j + r.
    b2q = np.zeros((128, 4), dtype=np.float32)
    for qq in range(4):
        for j in range(4):
            b2q[32 * j, qq] = b2[8 * qq + 2 * j, 0]
            b2q[32 * j + 1, qq] = b2[8 * qq + 2 * j + 1, 0]
    return l0t, l1t, l2t, b0t, b1t, b2q


def _run(x, mask_head_params, trace=False, trace_kwargs=None):
    from concourse.bass_utils import run_bass_kernel_spmd

    if "nc" not in _COMPILED:
        _COMPILED["nc"] = _build_program()
    nc = _COMPILED["nc"]

    x = np.ascontiguousarray(x, dtype=np.float32)
    l0t, l1t, l2t, b0t, b1t, b2q = _pack_params(mask_head_params)

    xx = np.tile(np.arange(W, dtype=np.float32) / W, HPC)  # [2000]
    in_maps = []
    for k in range(N_CORES):
        h0 = k * HPC
        yy = np.repeat((h0 + np.arange(HPC, dtype=np.float32)) / W, W)
        coords = np.stack([xx, yy], 0)  # [2, 2000]
        import ml_dtypes as _mld

        xsl = x[:, :, h0 : h0 + HPC, :].reshape(N_IMG, C, SPI)
        xs = np.ascontiguousarray(
            np.concatenate(
                [xsl, np.broadcast_to(coords, (N_IMG, 2, SPI))], axis=1
            ).astype(_mld.bfloat16)
        )
        in_maps.append(
            {
                "xs": xs,
                "l0t": l0t,
                "l1t": l1t,
                "l2t": l2t,
                "b0t": b0t,
                "b1t": b1t,
                "b2q": b2q,
            }
        )

    res = run_bass_kernel_spmd(
        nc,
        in_maps,
        list(range(N_CORES)),
        trace=trace,
        **(trace_kwargs or {}),
    )

    out = np.empty((1, M, H, W), dtype=np.float32)
    for k in range(N_CORES):
        oc = res.results[k]["out"].reshape(M, HPC, W)
        out[0, :, k * HPC : (k + 1) * HPC, :] = oc
    return out, res


def kernel(x, mask_head_params, num_ins):
    n_ins = int(np.asarray(num_ins))
    assert n_ins == NUM_INS, f"kernel hardcoded for num_ins={NUM_INS}, got {n_ins}"
    out, _ = _run(x, mask_head_params)
    return out



# revision 4
# speedup vs baseline: 1.7053x; 1.7053x over previous
"""Trainium2 Bass kernel for CondLaneRNNHead-style dynamic mask head.

Computation (see reference): per-instance 3-layer 1x1-conv MLP over
per-image feature maps augmented with 2 coordinate channels.

  out[m] = w2[m] @ relu(w1[m] @ relu(w0[m] @ [coords; x[img(m)]] + b0[m]) + b1[m]) + b2[m]

Shapes: x [4, 64, 80, 200] f32, mask_head_params [32, 8513] f32, num_ins=8.
Output [1, 32, 80, 200] f32.

Sharding: spatial, along H. Core k processes rows [10k, 10k+10) of all 4
images for all 32 instances. This replicates only the (small) per-instance
params across cores; the big x tensor is read exactly once in aggregate.

Device kernel structure (per core), v2 -- software-pipelined:
  - Instances packed in PAIRS (2 instances of one image) as in v1:
      layer0: lhsT [128(K: 66 real + pad), 128(M: 2x64 ch)], layer1 lhsT
      [128,128] block-diagonal, layer2 lhsT [128, 32] (cols 2-31 zero).
  - The PE clock boosts to 2.4 GHz only under sustained busy; any idle
    gap demotes it to 1.2 GHz (measured: gap-free windows ran matmuls at
    366 ns/512cols, windows with per-pair relu-wait gaps at ~600 ns).
    v1 emitted mm0,relu0,mm1,relu1,mm2 per pair, stalling the in-order PE
    ~0.5-1.0 us per pair waiting on relu. v2 software-pipelines with a
    3-deep skew: slot s issues mm0(u_s) and mm1(u_{s-2}); layer-2 matmuls
    for a quad-chunk are batched after its last pair's mm1, giving every
    relu ~2 slots of slack so the PE never waits.
  - relu work is spread over THREE engines (ACT, DVE, gpsimd) so the
    PE is the only saturated engine.
  - layer2 outputs of 4 pairs (a quad-chunk = 8 instances, one image)
    are packed into ONE PSUM tile at partition offsets {0,32,64,96}
    via matmul tile_position, so the PSUM->SBUF bias-move is 2 wide ops
    (halves on ACT + DVE, split at the 512 PSUM bank boundary so the
    next quad's first mm2 only waits on the first half) instead of 16
    thin ops. DMA cannot read PSUM on trn2.
  - Input DMAs are spread across all five engine queues, image-0 data
    first, so the first matmul can start as soon as the fixed ~8.5 us
    framework preamble ends; out DMAs alternate gpsimd/sync queues.
"""

import numpy as np
from contextlib import ExitStack

N_IMG, C, H, W = 4, 64, 80, 200
NUM_INS = 8
M = N_IMG * NUM_INS          # 32 instances
N_CORES = 8
HPC = H // N_CORES           # 10 rows of H per core
SPI = HPC * W                # 2000 spatial positions per image slice
PAIRS = M // 2               # 16
CH = C + 2                   # 66 input channels incl. coords
FD = 1000                    # activation chunk
# matmul free-dim splits inside each 1000 chunk: PSUM banks hold 512 f32, and
# a matmul output must not cross a bank boundary -> split 512 + 488.
SPLITS = ((0, 512), (512, 488))
FDP = 1024                   # padded stride in the quad PSUM tile

_W0N, _W1N, _W2N = CH * C, C * C, C
_B2_SHIFT = -2.19

_COMPILED = {}


def _build_program():
    import concourse.bacc as bacc
    import concourse.tile as tile
    from concourse import mybir

    dt = mybir.dt
    AF = mybir.ActivationFunctionType
    OP = mybir.AluOpType

    nc = bacc.Bacc("TRN2", target_bir_lowering=False, debug=False)

    # xs packs the 2 coordinate rows below the 64 x-channels so each image's
    # feats tile is filled by a single set of row-chunk DMAs.
    xs_d = nc.dram_tensor("xs", [N_IMG, CH, SPI], dt.bfloat16, kind="ExternalInput").ap()
    # layer0 lhsT zero-padded to K=128 (as v1: K=66 alone was measured to
    # keep the PE clock at 1.2 GHz).
    l0_d = nc.dram_tensor("l0t", [128, PAIRS * 128], dt.bfloat16, kind="ExternalInput").ap()
    l1_d = nc.dram_tensor("l1t", [128, PAIRS * 128], dt.bfloat16, kind="ExternalInput").ap()
    l2_d = nc.dram_tensor("l2t", [128, PAIRS * 32], dt.bfloat16, kind="ExternalInput").ap()
    b0_d = nc.dram_tensor("b0t", [128, PAIRS], dt.float32, kind="ExternalInput").ap()
    b1_d = nc.dram_tensor("b1t", [128, PAIRS], dt.float32, kind="ExternalInput").ap()
    b2_d = nc.dram_tensor("b2q", [128, 4], dt.float32, kind="ExternalInput").ap()
    # out[q, j, r, :] = instance 8q + 2j + r, i.e. plain instance-major order
    out_d = nc.dram_tensor("out", [4, 4, 2, SPI], dt.float32, kind="ExternalOutput").ap()

    f32 = dt.float32
    bf16 = dt.bfloat16

    with tile.TileContext(nc) as tc, ExitStack() as ctx:
        cpool = ctx.enter_context(tc.tile_pool(name="const", bufs=1))
        h1pool = ctx.enter_context(tc.tile_pool(name="h1p", bufs=4))
        h2pool = ctx.enter_context(tc.tile_pool(name="h2p", bufs=6))
        oqpool = ctx.enter_context(tc.tile_pool(name="oqp", bufs=4))
        pspool = ctx.enter_context(tc.tile_pool(name="ps", bufs=3, space="PSUM"))
        psqpool = ctx.enter_context(tc.tile_pool(name="psq", bufs=1, space="PSUM"))

        # ---- resident tiles ----
        b0s = cpool.tile([128, PAIRS], f32, tag="b0s", name="b0s")
        b1s = cpool.tile([128, PAIRS], f32, tag="b1s", name="b1s")
        b2s = cpool.tile([128, 4], f32, tag="b2s", name="b2s")
        l2s = cpool.tile([128, PAIRS * 32], bf16, tag="l2s", name="l2s")
        fe = [cpool.tile([128, SPI], bf16, tag=f"fe{n}", name=f"fe{n}")
              for n in range(N_IMG)]
        l0s = [cpool.tile([128, 4 * 128], bf16, tag=f"l0g{n}", name=f"l0g{n}")
               for n in range(N_IMG)]
        l1s = [cpool.tile([128, 4 * 128], bf16, tag=f"l1g{n}", name=f"l1g{n}")
               for n in range(N_IMG)]

        # ---- input loads, image-major, spread across the three DMA-capable
        # engine queues (gpsimd, sync/SP, scalar/Activation) ----
        # biases first (tiny; relu0 of slot 0 needs b0s)
        nc.sync.dma_start(b0s[:], b0_d[:])
        nc.sync.dma_start(b1s[:], b1_d[:])
        nc.sync.dma_start(b2s[:], b2_d[:])
        # vector zeroes the never-DMA'd pad rows (it cannot issue DMAs)
        for n in range(N_IMG):
            nc.vector.memset(fe[n][96:128, :], 0.0)
        for n in range(N_IMG):
            # fe pad rows 64:96 must be zero before the coord-row chunk
            # (rows 44:66 overlap rows 64-65) lands; same-queue order plus
            # the tile tracker's WAW dep guarantees it.
            nc.gpsimd.memset(fe[n][64:96, :], 0.0)
            nc.gpsimd.dma_start(fe[n][44:CH, :], xs_d[n, 44:CH, :])
            nc.sync.dma_start(fe[n][0:22, :], xs_d[n, 0:22, :])
            nc.scalar.dma_start(fe[n][22:44, :], xs_d[n, 22:44, :])
            nc.gpsimd.dma_start(l0s[n][:], l0_d[:, n * 512 : (n + 1) * 512])
            nc.scalar.dma_start(l1s[n][:], l1_d[:, n * 512 : (n + 1) * 512])
            nc.sync.dma_start(
                l2s[:, n * 128 : (n + 1) * 128], l2_d[:, n * 128 : (n + 1) * 128]
            )

        # ---- software-pipelined main loop ----
        # units: (image, chunk, pair-in-image), chunk-major inside an image
        # so a quad-chunk's four layer-2 batches are consecutive.
        units = [(i, h, j) for i in range(N_IMG) for h in range(2) for j in range(4)]
        U = len(units)
        h1t = [None] * U
        h2t = [None] * U
        ps0t = [None] * U
        ps1t = [None] * U

        # relu engine schedule: gpsimd cannot read PSUM, so ACT and DVE split
        # the relus. relu1(u) takes the opposite engine of relu0(u); since a
        # slot runs relu0(u_s) and relu1(u_{s-2}) (same parity), the two
        # relus of any slot land on different engines.
        def relu0_engine(s):
            return nc.scalar if s % 2 == 0 else nc.vector

        def relu1_engine(s):
            return nc.vector if s % 2 == 0 else nc.scalar

        def emit_relu(eng, dst, src, bias):
            if eng is nc.scalar:
                nc.scalar.activation(dst, src, AF.Relu, bias=bias)
            else:
                eng.tensor_scalar(dst, src, bias, 0.0, OP.add, OP.max)

        def stage0(s):
            i, h, j = units[s]
            p = 4 * i + j
            base = h * FD
            w0 = l0s[i][:, j * 128 : (j + 1) * 128]
            ps0 = pspool.tile([128, FD], f32, tag="ps", name=f"ps0_{s}")
            for off, sz in SPLITS:
                nc.tensor.matmul(
                    ps0[:, off : off + sz],
                    w0,
                    fe[i][:, base + off : base + off + sz],
                    start=True,
                    stop=True,
                )
            h1 = h1pool.tile([128, FD], bf16, tag="h1", name=f"h1_{s}")
            emit_relu(relu0_engine(s), h1[:], ps0[:], b0s[:, p : p + 1])
            h1t[s], ps0t[s] = h1, ps0

        def stage1(s):
            i, h, j = units[s]
            p = 4 * i + j
            w1 = l1s[i][:, j * 128 : (j + 1) * 128]
            ps1 = pspool.tile([128, FD], f32, tag="ps", name=f"ps1_{s}")
            for off, sz in SPLITS:
                nc.tensor.matmul(
                    ps1[:, off : off + sz],
                    w1,
                    h1t[s][:, off : off + sz],
                    start=True,
                    stop=True,
                )
            h2 = h2pool.tile([128, FD], bf16, tag="h2", name=f"h2_{s}")
            emit_relu(relu1_engine(s), h2[:], ps1[:], b1s[:, p : p + 1])
            h2t[s], ps1t[s] = h2, ps1

        def stage2_batch(i, h):
            # all 8 layer-2 matmuls of quad-chunk (i, h) in one burst, then
            # the bias-move halves and the output DMAs.
            base = h * FD
            psq = psqpool.tile([128, FDP], f32, tag="psq", name=f"psq_{i}_{h}")
            for j in range(4):
                s = 8 * i + 4 * h + j
                p = 4 * i + j
                w2 = l2s[:, 32 * p : 32 * p + 32]
                for off, sz in SPLITS:
                    nc.tensor.matmul(
                        psq[32 * j : 32 * j + 32, off : off + sz],
                        w2,
                        h2t[s][:, off : off + sz],
                        start=True,
                        stop=True,
                        tile_position=(0, 32 * j),
                    )
            oq = oqpool.tile([128, FD], f32, tag="oq", name=f"oq_{i}_{h}")
            # halves split at the 512 bank boundary: the next quad-chunk's
            # first mm2 (cols 0:512) only waits on the ACT half.
            nc.scalar.activation(
                oq[:, 0:512], psq[:, 0:512], AF.Identity, bias=b2s[:, i : i + 1]
            )
            nc.vector.tensor_scalar(
                oq[:, 512:FD], psq[:, 512:FD], b2s[:, i : i + 1], None, OP.add
            )
            dq = (nc.gpsimd, nc.sync) if (2 * i + h) % 2 == 0 else (nc.sync, nc.gpsimd)
            for j in range(4):
                dq[j % 2].dma_start(
                    out_d[i, j, :, base : base + FD], oq[32 * j : 32 * j + 2, :]
                )

        for s in range(U + 2):
            if s < U:
                stage0(s)
            if s >= 2:
                stage1(s - 2)
                i, h, j = units[s - 2]
                if j == 3:
                    stage2_batch(i, h)

    nc.compile()
    _dedupe_ldweights(nc, mybir)
    return nc


def _dedupe_ldweights(nc, mybir):
    """Drop redundant PE LDWEIGHTS after compile.

    Tile emits one LDWEIGHTS per matmul; consecutive matmuls here often share
    one stationary operand (split-column pairs), so the repeat loads only
    serialize the PE (~120ns each, and they block drain/fill overlap between
    back-to-back matmuls). Safe removal criteria: identical weights AP +
    tile_position as the last retained LDWEIGHTS (nothing between two
    LDWEIGHTS changes the loaded weights), and no semaphore waits/updates
    on the dropped instruction, so synchronization is untouched.
    """
    dropped = 0
    for fn in nc.m.functions:
        for blk in fn.blocks:
            new = []
            last_sig = None
            for i in blk.instructions:
                if (
                    isinstance(i, mybir.InstLdweights)
                    and i.engine == mybir.EngineType.PE
                ):
                    sig = (
                        str(i.ins[0]),
                        tuple(i.tile_position or ()),
                        i.perf_mode,
                        i.is_transpose,
                    )
                    si = i.sync_info
                    clean = si is None or (not si.on_wait and not si.on_update)
                    if clean and sig == last_sig:
                        dropped += 1
                        continue
                    last_sig = sig
                new.append(i)
            if dropped:
                blk.instructions.clear()
                blk.instructions.extend(new)
    return dropped


def _pack_params(mask_head_params):
    """Split generated params and build the pair-packed device layouts."""
    p = np.ascontiguousarray(mask_head_params, dtype=np.float32)
    o0, o1, o2 = _W0N, _W0N + _W1N, _W0N + _W1N + _W2N
    w0 = p[:, :o0].reshape(M, C, CH)
    w1 = p[:, o0:o1].reshape(M, C, C)
    w2 = p[:, o1:o2].reshape(M, C)
    b0 = p[:, o2 : o2 + C]
    b1 = p[:, o2 + C : o2 + 2 * C]
    b2 = p[:, o2 + 2 * C :] + np.float32(_B2_SHIFT)

    import ml_dtypes as _mld

    # layer0 lhsT [128, 16*128]: rows 0-63 = x-channel weights, 64-65 = coord
    # weights, 66-127 zero K-pad; cols pair-major then (inst a | inst b).
    w0T = np.transpose(w0, (2, 0, 1))              # [66(cin), 32, 64]
    w0T = np.concatenate([w0T[2:], w0T[:2]], 0)    # x channels first, coords last
    l0t = np.zeros((128, M * C), dtype=np.float32)
    l0t[:CH] = w0T.reshape(CH, M * C)
    l0t = np.ascontiguousarray(l0t.astype(_mld.bfloat16))

    l1 = np.zeros((PAIRS, 128, 128), dtype=np.float32)
    l1[:, :C, :C] = np.transpose(w1[0::2], (0, 2, 1))
    l1[:, C:, C:] = np.transpose(w1[1::2], (0, 2, 1))
    l1t = np.ascontiguousarray(
        np.transpose(l1, (1, 0, 2)).reshape(128, PAIRS * 128).astype(_mld.bfloat16)
    )

    l2 = np.zeros((PAIRS, 128, 32), dtype=np.float32)
    l2[:, :C, 0] = w2[0::2]
    l2[:, C:, 1] = w2[1::2]
    l2t = np.ascontiguousarray(
        np.transpose(l2, (1, 0, 2)).reshape(128, PAIRS * 32).astype(_mld.bfloat16)
    )

    b0t = np.ascontiguousarray(np.concatenate([b0[0::2], b0[1::2]], 1).T)  # [128,16]
    b1t = np.ascontiguousarray(np.concatenate([b1[0::2], b1[1::2]], 1).T)
    # b2 packed to match the quad PSUM layout: rows 32j+r of col q hold
    # instance 8q + 2j + r.
    b2q = np.zeros((128, 4), dtype=np.float32)
    for qq in range(4):
        for j in range(4):
            b2q[32 * j, qq] = b2[8 * qq + 2 * j, 0]
            b2q[32 * j + 1, qq] = b2[8 * qq + 2 * j + 1, 0]
    return l0t, l1t, l2t, b0t, b1t, b2q


def _run(x, mask_head_params, trace=False, trace_kwargs=None):
    from concourse.bass_utils import run_bass_kernel_spmd

    if "nc" not in _COMPILED:
        _COMPILED["nc"] = _build_program()
    nc = _COMPILED["nc"]

    x = np.ascontiguousarray(x, dtype=np.float32)
    l0t, l1t, l2t, b0t, b1t, b2q = _pack_params(mask_head_params)

    xx = np.tile(np.arange(W, dtype=np.float32) / W, HPC)  # [2000]
    in_maps = []
    for k in range(N_CORES):
        h0 = k * HPC
        yy = np.repeat((h0 + np.arange(HPC, dtype=np.float32)) / W, W)
        coords = np.stack([xx, yy], 0)  # [2, 2000]
        import ml_dtypes as _mld

        xsl = x[:, :, h0 : h0 + HPC, :].reshape(N_IMG, C, SPI)
        xs = np.ascontiguousarray(
            np.concatenate(
                [xsl, np.broadcast_to(coords, (N_IMG, 2, SPI))], axis=1
            ).astype(_mld.bfloat16)
        )
        in_maps.append(
            {
                "xs": xs,
                "l0t": l0t,
                "l1t": l1t,
                "l2t": l2t,
                "b0t": b0t,
                "b1t": b1t,
                "b2q": b2q,
            }
        )

    res = run_bass_kernel_spmd(
        nc,
        in_maps,
        list(range(N_CORES)),
        trace=trace,
        **(trace_kwargs or {}),
    )

    out = np.empty((1, M, H, W), dtype=np.float32)
    for k in range(N_CORES):
        oc = res.results[k]["out"].reshape(M, HPC, W)
        out[0, :, k * HPC : (k + 1) * HPC, :] = oc
    return out, res


def kernel(x, mask_head_params, num_ins):
    n_ins = int(np.asarray(num_ins))
    assert n_ins == NUM_INS, f"kernel hardcoded for num_ins={NUM_INS}, got {n_ins}"
    out, _ = _run(x, mask_head_params)
    return out
